# revision 5
# baseline (speedup 1.0000x reference)
"""MemoryNet kernel for 8 Trainium2 NeuronCores.

Math (per batch b):
    qn = q / ||q||_L2-over-L          (column-wise norm over sequence axis)
    kn = k / ||k||_L2-over-L
    qk[d, e] = sum_l qn[l, d] * kn[l, e]          # [D, D] channel cross-cov
    sm = softmax(qk, axis=e)
    out[l, d] = sum_e v[l, e] * sm[d, e]          # v @ sm^T

Key identity used: qk = (q^T k) * rnq[d] * rnk[e] where
    rnq[d] = 1/||q[:, d]||, rnk[e] = 1/||k[:, e]||
so normalization never touches the big [L, D] tensors.

Sharding (8 cores, B=4): core c -> batch b = c//2, L-half h = c%2.
Each core receives full q_b, k_b (needed for the full-L contraction) and
its half of v_b; computes its half of out_b.  No collectives.

Since |qk| <= 1 (dot products of unit vectors), softmax is computed
without max-subtraction.  The reference's max(norm, 1e-12) clamp is a
no-op for these magnitudes (norms ~sqrt(2048)).
"""

import numpy as np

import concourse.bass as bass
import concourse.bacc as bacc
import concourse.mybir as mybir
import concourse.tile as tile
from concourse.bass_utils import run_bass_kernel_spmd
from concourse.masks import make_identity

F32 = mybir.dt.float32
B, L, D = 4, 2048, 128
P = 128                    # SBUF partitions
NCORES = 8
LV = L // 2                # v/out rows per core
NT = L // P                # 16 q/k L-tiles per core
NVT = LV // P              # 8 v L-tiles per core
TPC = 4                    # L-tiles per DMA chunk
NCHUNK = NT // TPC         # 4 q/k chunks
VCHUNK = NVT // TPC        # 2 v chunks


def _build() -> bass.Bass:
    nc = bacc.Bacc("TRN2", target_bir_lowering=False, debug=False)
    q_d = nc.dram_tensor("q", [L, D], F32, kind="ExternalInput")
    k_d = nc.dram_tensor("k", [L, D], F32, kind="ExternalInput")
    v_d = nc.dram_tensor("v", [LV, D], F32, kind="ExternalInput")
    o_d = nc.dram_tensor("out", [LV, D], F32, kind="ExternalOutput")

    q_r = q_d.rearrange("(t p) d -> p t d", p=P)   # [128, 16, 128]
    k_r = k_d.rearrange("(t p) d -> p t d", p=P)
    v_r = v_d.rearrange("(t p) d -> p t d", p=P)   # [128, 8, 128]
    o_r = o_d.rearrange("(t p) d -> p t d", p=P)

    with tile.TileContext(nc) as tc:
        with (
            tc.tile_pool(name="persist", bufs=1) as persist,
            tc.tile_pool(name="chunks", bufs=3) as chunks,
            tc.tile_pool(name="work", bufs=2) as work,
            tc.tile_pool(name="ps_acc", bufs=1, space="PSUM") as ps_acc,
            tc.tile_pool(name="ps_mm", bufs=2, space="PSUM") as ps_mm,
        ):
            ident = persist.tile([P, P], F32)
            make_identity(nc, ident)
            ones_col = persist.tile([P, 1], F32)
            nc.vector.memset(ones_col, 1.0)
            ones_row = persist.tile([1, P], F32)
            nc.vector.memset(ones_row, 1.0)

            # ---- phase 1: qk_raw = q^T k, qq = q^T q, sq_k = ones^T (k*k) ----
            # accumulated over 16 L-tiles; q/k stream through chunked tiles
            ps_big = ps_acc.tile([P, 2 * P], F32)   # [:, :128]=qk  [:, 128:]=qq
            ps_sqk = ps_acc.tile([1, P], F32)       # column sums of k*k

            # k and q share one chunk tile so qk|qq is a single [128,256]
            # matmul accumulation group (interleaved groups in one PSUM bank
            # corrupt each other: the start=True clear is bank-granular).
            kq = [None] * NCHUNK
            for j in range(NCHUNK):
                kq[j] = chunks.tile([P, TPC, 2 * D], F32, tag="kq", name=f"kq{j}")
                nc.sync.dma_start(out=kq[j][:, :, 0:D],
                                  in_=k_r[:, j * TPC:(j + 1) * TPC, :])
                nc.sync.dma_start(out=kq[j][:, :, D:2 * D],
                                  in_=q_r[:, j * TPC:(j + 1) * TPC, :])

            for t in range(NT):
                j, s = divmod(t, TPC)
                qt = kq[j][:, s, D:2 * D]
                kqt = kq[j][:, s, :]
                k2 = work.tile([P, D], F32, tag="k2")
                nc.scalar.activation(out=k2, in_=kq[j][:, s, 0:D],
                                     func=mybir.ActivationFunctionType.Square)
                st, sp = (t == 0), (t == NT - 1)
                nc.tensor.matmul(ps_big, lhsT=qt, rhs=kqt, start=st, stop=sp)
                nc.tensor.matmul(ps_sqk, lhsT=ones_col, rhs=k2, start=st, stop=sp)

            # ---- v loads + transposes (independent of softmax path) ----
            vc = [None] * VCHUNK
            for j in range(VCHUNK):
                vc[j] = chunks.tile([P, TPC, D], F32, tag="vc", name=f"vc{j}")
                nc.sync.dma_start(out=vc[j], in_=v_r[:, j * TPC:(j + 1) * TPC, :])
            sb_vT = persist.tile([P, NVT, D], F32)  # v^T tiles: [e, t, l]
            for t in range(NVT):
                j, s = divmod(t, TPC)
                ps_vT = ps_mm.tile([P, P], F32, tag="vT")
                nc.tensor.transpose(ps_vT, vc[j][:, s, :], ident)
                nc.vector.tensor_copy(sb_vT[:, t, :], ps_vT)

            # ---- softmax over e (free axis), fused normalization ----
            # sq_q = diag(qq)
            dtmp = work.tile([P, P], F32)
            nc.vector.tensor_mul(dtmp, ps_big[:, P:2 * P], ident)
            sq_q = work.tile([P, 1], F32)
            nc.vector.reduce_sum(sq_q, dtmp, axis=mybir.AxisListType.X)
            nq = work.tile([P, 1], F32)
            nc.scalar.activation(out=nq, in_=sq_q,
                                 func=mybir.ActivationFunctionType.Sqrt)
            rnq = work.tile([P, 1], F32)
            nc.vector.reciprocal(rnq, nq)

            nk_row = work.tile([1, P], F32)
            nc.scalar.activation(out=nk_row, in_=ps_sqk,
                                 func=mybir.ActivationFunctionType.Sqrt)
            rnk_row = work.tile([1, P], F32)
            nc.vector.reciprocal(rnk_row, nk_row)

            # broadcast rnk over partitions: ones_row^T @ rnk_row -> [128,128]
            ps_b = ps_mm.tile([P, P], F32, tag="bc", bufs=1)
            nc.tensor.matmul(ps_b, lhsT=ones_row, rhs=rnk_row, start=True, stop=True)
            rnk_b = work.tile([P, P], F32)
            nc.vector.tensor_copy(rnk_b, ps_b)

            qks = work.tile([P, P], F32)
            nc.vector.tensor_mul(qks, ps_big[:, 0:P], rnk_b)
            # E = exp(rnq * qks); S[d] = sum_e E[d, e]  (fused row-sum)
            E = work.tile([P, P], F32)
            S = work.tile([P, 1], F32)
            nc.scalar.activation(out=E, in_=qks,
                                 func=mybir.ActivationFunctionType.Exp,
                                 scale=rnq, accum_out=S)
            rS = work.tile([P, 1], F32)
            nc.vector.reciprocal(rS, S)
            sm = work.tile([P, P], F32)
            nc.vector.tensor_scalar_mul(sm, E, rS)

            ps_smT = ps_mm.tile([P, P], F32, tag="smT", bufs=1)
            nc.tensor.transpose(ps_smT, sm, ident)
            smT = persist.tile([P, P], F32)   # [e, d]
            nc.vector.tensor_copy(smT, ps_smT)

            # ---- phase 2: out[l, d] = sum_e vT[e, l] * smT[e, d] ----
            sb_out = persist.tile([P, NVT, D], F32)
            for t in range(NVT):
                ps_o = ps_mm.tile([P, P], F32, tag="po")
                nc.tensor.matmul(ps_o, lhsT=sb_vT[:, t, :], rhs=smT,
                                 start=True, stop=True)
                nc.vector.tensor_copy(sb_out[:, t, :], ps_o)
                if t % TPC == TPC - 1:
                    j = t // TPC
                    nc.sync.dma_start(out=o_r[:, j * TPC:(j + 1) * TPC, :],
                                      in_=sb_out[:, j * TPC:(j + 1) * TPC, :])
    nc.compile()
    return nc


_CACHE: dict = {}


def _get_nc() -> bass.Bass:
    if "nc" not in _CACHE:
        _CACHE["nc"] = _build()
    return _CACHE["nc"]


def kernel(q: np.ndarray, k: np.ndarray, v: np.ndarray) -> np.ndarray:
    nc = _get_nc()
    q = np.ascontiguousarray(np.asarray(q, dtype=np.float32))
    k = np.ascontiguousarray(np.asarray(k, dtype=np.float32))
    v = np.ascontiguousarray(np.asarray(v, dtype=np.float32))
    in_maps = []
    for c in range(NCORES):
        b, h = divmod(c, 2)
        in_maps.append({
            "q": q[b],
            "k": k[b],
            "v": np.ascontiguousarray(v[b, h * LV:(h + 1) * LV]),
        })
    res = run_bass_kernel_spmd(nc, in_maps, list(range(NCORES))).results
    out = np.empty((B, L, D), dtype=np.float32)
    for c in range(NCORES):
        b, h = divmod(c, 2)
        out[b, h * LV:(h + 1) * LV] = res[c]["out"]
    return out


# revision 7
# speedup vs baseline: 1.2196x; 1.2196x over previous
"""MemoryNet kernel for 8 Trainium2 NeuronCores.

Math (per batch b):
    qn = q / ||q||_L2-over-L          (column-wise norm over sequence axis)
    kn = k / ||k||_L2-over-L
    qk[d, e] = sum_l qn[l, d] * kn[l, e]          # [D, D] channel cross-cov
    sm = softmax(qk, axis=e)
    out[l, d] = sum_e v[l, e] * sm[d, e]          # v @ sm^T

Key identity: qk = (q^T k) * rnq[d] * rnk[e] with rnq = 1/||q[:,d]||,
rnk = 1/||k[:,e]|| — normalization never touches the big [L, D] tensors.
sq_q comes from diag(q^T q), sq_k from diag(k^T k), both computed on the
PE alongside q^T k.

Sharding (8 cores, B=4): core c -> batch b = c//2, L-half h = c%2.
Each core receives full q_b, k_b (needed for the full-L contraction) and
its half of v_b; computes its half of out_b.  No collectives.

Layout trick: HBM rows are only 512B, so a [l-on-partitions] tile load
would use 512B DMA descriptors (4x off line rate).  Instead each SBUF
partition p holds CONSECUTIVE HBM rows (16 for q/k, 8 for v/out), giving
4-8KB contiguous descriptors.  The L-contraction is order-free, so
matmul L-"tiles" are the interleaved row sets {16p + t}; accumulating
over t=0..15 still sums over all L exactly.

Precision: q/k are cast to bf16 during DMA (SWDGE) — they only feed the
softmax logits, where |logits| <= 1 and the bf16-level error (~2e-4
relative on the logits) is far below fp32 output tolerance.  The v-path
(v transposes + v @ sm^T) stays full fp32 (PE fp32 = exact 2-pass mode).

Since |qk| <= 1, softmax runs without max-subtraction.  The reference's
max(norm, 1e-12) clamp is a no-op at these magnitudes (norms ~sqrt(2048)).
"""

import numpy as np

import concourse.bass as bass
import concourse.bacc as bacc
import concourse.mybir as mybir
import concourse.tile as tile
from concourse.bass_utils import run_bass_kernel_spmd
from concourse.masks import make_identity

F32 = mybir.dt.float32
BF16 = mybir.dt.bfloat16
B, L, D = 4, 2048, 128
P = 128                    # SBUF partitions
NCORES = 8
LV = L // 2                # v/out rows per core
NT = L // P                # 16 q/k L-groups per core
NVT = LV // P              # 8 v L-groups per core
TPC = 4                    # L-groups per DMA chunk (q/k)
NCHUNK = NT // TPC         # 4 q/k chunks


def _build() -> bass.Bass:
    nc = bacc.Bacc("TRN2", target_bir_lowering=False, debug=False)
    q_d = nc.dram_tensor("q", [L, D], F32, kind="ExternalInput")
    k_d = nc.dram_tensor("k", [L, D], F32, kind="ExternalInput")
    v_d = nc.dram_tensor("v", [LV, D], F32, kind="ExternalInput")
    o_d = nc.dram_tensor("out", [LV, D], F32, kind="ExternalOutput")

    # flat views: partition p <- consecutive HBM rows (big DMA descriptors)
    q_r = q_d.rearrange("(p t) d -> p t d", p=P)   # [128, 16, 128], row 16p+t
    k_r = k_d.rearrange("(p t) d -> p t d", p=P)
    v_r = v_d.rearrange("(p s) d -> p s d", p=P)   # [128, 8, 128], row 8p+s
    o_r = o_d.rearrange("(p s) d -> p s d", p=P)

    with tile.TileContext(nc) as tc:
        with (
            tc.tile_pool(name="persist", bufs=1) as persist,
            tc.tile_pool(name="work", bufs=2) as work,
            tc.tile_pool(name="ps_acc", bufs=1, space="PSUM") as ps_acc,
            tc.tile_pool(name="ps_mid", bufs=1, space="PSUM") as ps_mid,
            tc.tile_pool(name="ps_mm", bufs=2, space="PSUM") as ps_mm,
        ):
            ident = persist.tile([P, P], F32)
            make_identity(nc, ident)
            ones_row = persist.tile([1, P], F32)
            nc.vector.memset(ones_row, 1.0)

            # ---- loads ----
            # q/k: bf16 via SWDGE cast-DMA, into one [128, 16, 256] tile so
            # [k_t | q_t] is a single contiguous 256-wide matmul rhs.
            sb_kq = persist.tile([P, NT, 2 * D], BF16)
            for j in range(NCHUNK):
                sl = slice(j * TPC, (j + 1) * TPC)
                nc.gpsimd.dma_start(out=sb_kq[:, sl, 0:D], in_=k_r[:, sl, :])
                nc.gpsimd.dma_start(out=sb_kq[:, sl, D:2 * D], in_=q_r[:, sl, :])
            # v: fp32 exact via HWDGE
            sb_v = persist.tile([P, NVT, D], F32)
            nc.sync.dma_start(out=sb_v[:, 0:NVT // 2, :], in_=v_r[:, 0:NVT // 2, :])
            nc.sync.dma_start(out=sb_v[:, NVT // 2:, :], in_=v_r[:, NVT // 2:, :])

            # ---- phase 1 (PE): ps_A = [q^T k | q^T q], ps_kk = k^T k ----
            ps_A = ps_acc.tile([P, 2 * D], F32)
            ps_kk = ps_acc.tile([P, D], F32)
            for t in range(NT):
                qt = sb_kq[:, t, D:2 * D]
                kt = sb_kq[:, t, 0:D]
                st, sp = (t == 0), (t == NT - 1)
                nc.tensor.matmul(ps_A, lhsT=qt, rhs=sb_kq[:, t, :],
                                 start=st, stop=sp)
                nc.tensor.matmul(ps_kk, lhsT=kt, rhs=kt, start=st, stop=sp)

            # ---- v transposes (PE, fp32 exact): vT[:, s, :] = v_s^T ----
            sb_vT = persist.tile([P, NVT, D], F32)
            for s in range(NVT):
                ps_vT = ps_mm.tile([P, P], F32, tag="vT")
                nc.tensor.transpose(ps_vT, sb_v[:, s, :], ident)
                nc.vector.tensor_copy(sb_vT[:, s, :], ps_vT)

            # ---- norms ----
            # sq = diag of the gram blocks: (gram * I) summed along free axis
            dq = work.tile([P, P], F32)
            nc.vector.tensor_mul(dq, ps_A[:, D:2 * D], ident)
            sq_q = work.tile([P, 1], F32)
            nc.vector.reduce_sum(sq_q, dq, axis=mybir.AxisListType.X)
            dk = work.tile([P, P], F32)
            nc.vector.tensor_mul(dk, ps_kk, ident)
            sq_k = work.tile([P, 1], F32)
            nc.vector.reduce_sum(sq_k, dk, axis=mybir.AxisListType.X)
            nq = work.tile([P, 1], F32)
            nc.scalar.activation(out=nq, in_=sq_q,
                                 func=mybir.ActivationFunctionType.Sqrt)
            rnq = work.tile([P, 1], F32)
            nc.vector.reciprocal(rnq, nq)
            nk = work.tile([P, 1], F32)
            nc.scalar.activation(out=nk, in_=sq_k,
                                 func=mybir.ActivationFunctionType.Sqrt)
            rnk = work.tile([P, 1], F32)
            nc.vector.reciprocal(rnk, nk)

            # rnk as a broadcast matrix: transpose to a row, outer with ones
            ps_rT = ps_mid.tile([1, P], F32, tag="mid", name="ps_rT")
            nc.tensor.transpose(ps_rT, rnk, ident)
            rnk_row = work.tile([1, P], F32)
            nc.vector.tensor_copy(rnk_row, ps_rT)
            ps_bc = ps_mid.tile([P, P], F32, tag="mid", name="ps_bc")
            nc.tensor.matmul(ps_bc, lhsT=ones_row, rhs=rnk_row,
                             start=True, stop=True)
            rnk_b = work.tile([P, P], F32)
            nc.vector.tensor_copy(rnk_b, ps_bc)

            # ---- softmax over e (free axis) ----
            qks = work.tile([P, P], F32)
            nc.vector.tensor_mul(qks, ps_A[:, 0:D], rnk_b)
            E = work.tile([P, P], F32)
            S = work.tile([P, 1], F32)
            nc.scalar.activation(out=E, in_=qks,
                                 func=mybir.ActivationFunctionType.Exp,
                                 scale=rnq, accum_out=S)
            rS = work.tile([P, 1], F32)
            nc.vector.reciprocal(rS, S)
            sm = work.tile([P, P], F32)
            nc.vector.tensor_scalar_mul(sm, E, rS)
            ps_smT = ps_mid.tile([P, P], F32, tag="mid", name="ps_smT")
            nc.tensor.transpose(ps_smT, sm, ident)
            smT = persist.tile([P, P], F32)   # [e, d]
            nc.vector.tensor_copy(smT, ps_smT)

            # ---- phase 2 (PE, fp32): out_s[l, d] = vT_s^T @ smT ----
            sb_out = persist.tile([P, NVT, D], F32)
            for s in range(NVT):
                ps_o = ps_mm.tile([P, P], F32, tag="po")
                nc.tensor.matmul(ps_o, lhsT=sb_vT[:, s, :], rhs=smT,
                                 start=True, stop=True)
                nc.vector.tensor_copy(sb_out[:, s, :], ps_o)
                if s == NVT // 2 - 1:
                    nc.scalar.dma_start(out=o_r[:, 0:NVT // 2, :],
                                        in_=sb_out[:, 0:NVT // 2, :])
                elif s == NVT - 1:
                    nc.scalar.dma_start(out=o_r[:, NVT // 2:, :],
                                        in_=sb_out[:, NVT // 2:, :])
    nc.compile()
    return nc


_CACHE: dict = {}


def _get_nc() -> bass.Bass:
    if "nc" not in _CACHE:
        _CACHE["nc"] = _build()
    return _CACHE["nc"]


def kernel(q: np.ndarray, k: np.ndarray, v: np.ndarray) -> np.ndarray:
    nc = _get_nc()
    q = np.ascontiguousarray(np.asarray(q, dtype=np.float32))
    k = np.ascontiguousarray(np.asarray(k, dtype=np.float32))
    v = np.ascontiguousarray(np.asarray(v, dtype=np.float32))
    in_maps = []
    for c in range(NCORES):
        b, h = divmod(c, 2)
        in_maps.append({
            "q": q[b],
            "k": k[b],
            "v": np.ascontiguousarray(v[b, h * LV:(h + 1) * LV]),
        })
    res = run_bass_kernel_spmd(nc, in_maps, list(range(NCORES))).results
    out = np.empty((B, L, D), dtype=np.float32)
    for c in range(NCORES):
        b, h = divmod(c, 2)
        out[b, h * LV:(h + 1) * LV] = res[c]["out"]
    return out


# revision 9
# speedup vs baseline: 1.3618x; 1.1166x over previous
"""MemoryNet kernel for 8 Trainium2 NeuronCores.

Math (per batch b):
    qn = q / ||q||_L2-over-L          (column-wise norm over sequence axis)
    kn = k / ||k||_L2-over-L
    qk[d, e] = sum_l qn[l, d] * kn[l, e]          # [D, D] channel cross-cov
    sm = softmax(qk, axis=e)
    out[l, d] = sum_e v[l, e] * sm[d, e]          # v @ sm^T

Key identity: qk = (q^T k) * rnq[d] * rnk[e] with rnq = 1/||q[:,d]||,
rnk = 1/||k[:,e]|| — normalization never touches the big [L, D] tensors.
sq_q comes from diag(q^T q), sq_k from diag(k^T k), both computed on the
PE alongside q^T k.

Sharding (8 cores, B=4): core c -> batch b = c//2, L-half h = c%2.
Each core receives full q_b, k_b (needed for the full-L contraction) and
its half of v_b; computes its half of out_b.  No collectives.

Layout trick: HBM rows are only 512B, so a [l-on-partitions] tile load
would use 512B DMA descriptors (4x off line rate).  Instead each SBUF
partition p holds CONSECUTIVE HBM rows (16 for q/k, 8 for v/out), giving
4-8KB contiguous descriptors.  The L-contraction is order-free, so
matmul L-"tiles" are the interleaved row sets {16p + t}; accumulating
over t=0..15 still sums over all L exactly.

Precision: q/k are cast to fp16 on the host — they only feed the
softmax logits, where |logits| <= 1; fp16's 11-bit mantissa keeps the
logit error ~1e-5, far below fp32 output tolerance, and halves q/k HBM
traffic.  The v-path (v transposes + v @ sm^T) stays full fp32 (PE fp32
= exact 2-pass mode).

Since |qk| <= 1, softmax runs without max-subtraction.  The reference's
max(norm, 1e-12) clamp is a no-op at these magnitudes (norms ~sqrt(2048)).
"""

import numpy as np

import concourse.bass as bass
import concourse.bacc as bacc
import concourse.mybir as mybir
import concourse.tile as tile
from concourse.bass_utils import run_bass_kernel_spmd
from concourse.masks import make_identity

F32 = mybir.dt.float32
F16 = mybir.dt.float16
B, L, D = 4, 2048, 128
P = 128                    # SBUF partitions
NCORES = 8
LV = L // 2                # v/out rows per core
NT = L // P                # 16 q/k L-groups per core
NVT = LV // P              # 8 v L-groups per core
TPC = 4                    # L-groups per DMA chunk (q/k)
NCHUNK = NT // TPC         # 4 q/k chunks


def _build() -> bass.Bass:
    nc = bacc.Bacc("TRN2", target_bir_lowering=False, debug=False)
    q_d = nc.dram_tensor("q", [L, D], F16, kind="ExternalInput")
    k_d = nc.dram_tensor("k", [L, D], F16, kind="ExternalInput")
    v_d = nc.dram_tensor("v", [LV, D], F32, kind="ExternalInput")
    o_d = nc.dram_tensor("out", [LV, D], F32, kind="ExternalOutput")

    # flat views: partition p <- consecutive HBM rows (big DMA descriptors)
    q_r = q_d.rearrange("(p t) d -> p t d", p=P)   # [128, 16, 128], row 16p+t
    k_r = k_d.rearrange("(p t) d -> p t d", p=P)
    v_r = v_d.rearrange("(p s) d -> p s d", p=P)   # [128, 8, 128], row 8p+s
    o_r = o_d.rearrange("(p s) d -> p s d", p=P)

    with tile.TileContext(nc) as tc:
        with (
            tc.tile_pool(name="persist", bufs=1) as persist,
            tc.tile_pool(name="work", bufs=2) as work,
            tc.tile_pool(name="ps_acc", bufs=1, space="PSUM") as ps_acc,
            tc.tile_pool(name="ps_mid", bufs=1, space="PSUM") as ps_mid,
            tc.tile_pool(name="ps_mm", bufs=2, space="PSUM") as ps_mm,
        ):
            ident = persist.tile([P, P], F32)
            make_identity(nc, ident)
            ones_row = persist.tile([1, P], F32)
            nc.vector.memset(ones_row, 1.0)

            # ---- loads (both HWDGE rings in parallel) ----
            # q on the SP ring, k on the ACT ring; 4KB/partition descriptors
            sb_q = persist.tile([P, NT, D], F16)
            sb_k = persist.tile([P, NT, D], F16)
            for j in range(NCHUNK):
                sl = slice(j * TPC, (j + 1) * TPC)
                nc.scalar.dma_start(out=sb_k[:, sl, :], in_=k_r[:, sl, :])
                nc.sync.dma_start(out=sb_q[:, sl, :], in_=q_r[:, sl, :])
            # v: fp32 exact
            sb_v = persist.tile([P, NVT, D], F32)
            nc.sync.dma_start(out=sb_v[:, 0:NVT // 2, :], in_=v_r[:, 0:NVT // 2, :])
            nc.sync.dma_start(out=sb_v[:, NVT // 2:, :], in_=v_r[:, NVT // 2:, :])

            # ---- phase 1 (PE): q^T k, q^T q, k^T k ----
            # one PSUM bank per accumulation group: a start=True clear is
            # bank-granular and wipes a sibling group's has_written bits
            ps_qk = ps_acc.tile([P, D], F32)
            ps_qq = ps_acc.tile([P, D], F32)
            ps_kk = ps_acc.tile([P, D], F32)
            for t in range(NT):
                qt = sb_q[:, t, :]
                kt = sb_k[:, t, :]
                st, sp = (t == 0), (t == NT - 1)
                nc.tensor.matmul(ps_qk, lhsT=qt, rhs=kt, start=st, stop=sp)
                nc.tensor.matmul(ps_qq, lhsT=qt, rhs=qt, start=st, stop=sp)
                nc.tensor.matmul(ps_kk, lhsT=kt, rhs=kt, start=st, stop=sp)

            # ---- v transposes (PE, fp32 exact): vT[:, s, :] = v_s^T ----
            sb_vT = persist.tile([P, NVT, D], F32)
            for s in range(NVT):
                ps_vT = ps_mm.tile([P, P], F32, tag="vT")
                nc.tensor.transpose(ps_vT, sb_v[:, s, :], ident)
                nc.vector.tensor_copy(sb_vT[:, s, :], ps_vT)

            # ---- norms ----
            # sq = diag of the gram blocks: (gram * I) summed along free axis
            dq = work.tile([P, P], F32)
            nc.vector.tensor_mul(dq, ps_qq, ident)
            sq_q = work.tile([P, 1], F32)
            nc.vector.reduce_sum(sq_q, dq, axis=mybir.AxisListType.X)
            dk = work.tile([P, P], F32)
            nc.vector.tensor_mul(dk, ps_kk, ident)
            sq_k = work.tile([P, 1], F32)
            nc.vector.reduce_sum(sq_k, dk, axis=mybir.AxisListType.X)
            nq = work.tile([P, 1], F32)
            nc.scalar.activation(out=nq, in_=sq_q,
                                 func=mybir.ActivationFunctionType.Sqrt)
            rnq = work.tile([P, 1], F32)
            nc.vector.reciprocal(rnq, nq)
            nk = work.tile([P, 1], F32)
            nc.scalar.activation(out=nk, in_=sq_k,
                                 func=mybir.ActivationFunctionType.Sqrt)
            rnk = work.tile([P, 1], F32)
            nc.vector.reciprocal(rnk, nk)

            # rnk as a broadcast matrix: transpose to a row, outer with ones
            ps_rT = ps_mid.tile([1, P], F32, tag="mid", name="ps_rT")
            nc.tensor.transpose(ps_rT, rnk, ident)
            rnk_row = work.tile([1, P], F32)
            nc.vector.tensor_copy(rnk_row, ps_rT)
            ps_bc = ps_mid.tile([P, P], F32, tag="mid", name="ps_bc")
            nc.tensor.matmul(ps_bc, lhsT=ones_row, rhs=rnk_row,
                             start=True, stop=True)
            rnk_b = work.tile([P, P], F32)
            nc.vector.tensor_copy(rnk_b, ps_bc)

            # ---- softmax over e (free axis) ----
            qks = work.tile([P, P], F32)
            nc.vector.tensor_mul(qks, ps_qk, rnk_b)
            E = work.tile([P, P], F32)
            S = work.tile([P, 1], F32)
            nc.scalar.activation(out=E, in_=qks,
                                 func=mybir.ActivationFunctionType.Exp,
                                 scale=rnq, accum_out=S)
            rS = work.tile([P, 1], F32)
            nc.vector.reciprocal(rS, S)
            sm = work.tile([P, P], F32)
            nc.vector.tensor_scalar_mul(sm, E, rS)
            ps_smT = ps_mid.tile([P, P], F32, tag="mid", name="ps_smT")
            nc.tensor.transpose(ps_smT, sm, ident)
            smT = persist.tile([P, P], F32)   # [e, d]
            nc.vector.tensor_copy(smT, ps_smT)

            # ---- phase 2 (PE, fp32): out_s[l, d] = vT_s^T @ smT ----
            sb_out = persist.tile([P, NVT, D], F32)
            for s in range(NVT):
                ps_o = ps_mm.tile([P, P], F32, tag="po")
                nc.tensor.matmul(ps_o, lhsT=sb_vT[:, s, :], rhs=smT,
                                 start=True, stop=True)
                nc.vector.tensor_copy(sb_out[:, s, :], ps_o)
                if s == NVT // 2 - 1:
                    nc.scalar.dma_start(out=o_r[:, 0:NVT // 2, :],
                                        in_=sb_out[:, 0:NVT // 2, :])
                elif s == NVT - 1:
                    nc.scalar.dma_start(out=o_r[:, NVT // 2:, :],
                                        in_=sb_out[:, NVT // 2:, :])
    nc.compile()
    return nc


_CACHE: dict = {}


def _get_nc() -> bass.Bass:
    if "nc" not in _CACHE:
        _CACHE["nc"] = _build()
    return _CACHE["nc"]


def kernel(q: np.ndarray, k: np.ndarray, v: np.ndarray) -> np.ndarray:
    nc = _get_nc()
    q = np.ascontiguousarray(np.asarray(q, dtype=np.float32).astype(np.float16))
    k = np.ascontiguousarray(np.asarray(k, dtype=np.float32).astype(np.float16))
    v = np.ascontiguousarray(np.asarray(v, dtype=np.float32))
    in_maps = []
    for c in range(NCORES):
        b, h = divmod(c, 2)
        in_maps.append({
            "q": q[b],
            "k": k[b],
            "v": np.ascontiguousarray(v[b, h * LV:(h + 1) * LV]),
        })
    res = run_bass_kernel_spmd(nc, in_maps, list(range(NCORES))).results
    out = np.empty((B, L, D), dtype=np.float32)
    for c in range(NCORES):
        b, h = divmod(c, 2)
        out[b, h * LV:(h + 1) * LV] = res[c]["out"]
    return out
